# revision 1
# baseline (speedup 1.0000x reference)
"""DeepFourierTransform kernel for Trainium2 (8 NeuronCores, data-parallel).

Problem:
  x [4096, 4096] f32 -> sliding windows (31 per row, size 256, hop 128)
  cos_feat = cos(win @ w_cos.T + b_cos)   [B, 31, 512]
  sin_feat = sin(win @ w_sin.T + b_sin)   [B, 31, 512]
  out = concat(cos,sin) @ w_out.T + b_out, mean over windows, log_softmax
  -> [4096, 4] f32

Strategy (per core, batch shard of 512 rows):
  - Main matmuls in fp8-e4m3 DoubleRow perf mode: K=256 (one window) as 2
    k-tiles of 128 in ONE matmul at 0.5 cycles/row -> PE main cost ~27us
    (vs ~107us bf16).  fp8 quantization noise washes out over the
    31x1024-feature mean (validated: end-to-end L2 ~1.4e-3 vs 2e-2 gate).
  - Window accumulation: instead of DVE tree-adds (~77us), every window's
    feature tile is immediately projected onto the 4 outputs by tiny PE
    matmuls (lhsT = 128x128 feat block, rhs = w_out.T/31 [128,4], out
    [128,4]) accumulating into one persistent PSUM bank (memset once,
    start=False).  992 matmuls x ~1.7ns engine time.
  - The 16.25M Sin/Cos evals split across two engines:
      * ACT: Sin with fused per-partition bias, 0.833ns/elem.
      * DVE: degree-6 factored polynomial for cos combos,
        cos(v) ~ cc*(u - r1)*((u - Re)^2 + Im2), u = v^2:
        1 TT mult @1x (PSUM f32 read) + 3 tensor_scalar @4x + 2 TT @2x
        ~ 2.9ns/elem.  Bias is pre-added into PSUM by a K=1 bf16 matmul
        (b_cos row x ones) so the poly runs on biased z directly.
  - PSUM: tile A [128,4,512] (4 banks) + tile B [128,3,512] (3 banks)
    ping-pong at chunk granularity + 1 bank for the output accumulator.
    Slot order: lone 3-window block first (combos alternate A/B), then 4
    pairs of (4w on A, 3w on B) per combo, cos/sin interleaved so the 16
    DVE chunks (pair-B, cos combos) are spaced 4 slots apart.
  - Projections for slot s are emitted after mains of slot s+2 so PE never
    head-of-line blocks on a feat tile.
  - Tail: z = pf + b_out, batched log_softmax (no max-shift: |z|<=~3).
  - Exp/Ln steered to the shared natural_log_exp table set; a warmup Sin
    pulls the trig table load to t~0; dummy matmuls pre-warm the PE clock.
"""

import numpy as np
import ml_dtypes

import concourse.bass as bass
import concourse.bacc as bacc
import concourse.mybir as mybir
import concourse.tile as tile
from concourse.bass_utils import run_bass_kernel_spmd

BF16 = mybir.dt.bfloat16
F32 = mybir.dt.float32
FP8 = mybir.dt.float8e4

N_CORES = 8
B = 4096
B_LOCAL = B // N_CORES          # 512
SEQ = 4096
P = 128
NCHUNK = SEQ // P               # 32
NWIN = 31
M = 512                         # features per trig branch
NCOMBO = 8                      # 4 cos m-tiles + 4 sin m-tiles
OUT_DIM = 4
NBT = B_LOCAL // P              # 4 batch tiles of 128

# degree-6 (in v) minimax-ish polynomial for cos(v), |v| <= 3.85:
#   cos(v) ~ CC*(u - R1)*((u - RE)^2 + IM2),  u = v^2
# max err 6.1e-3 on the range; fitted offline.
CC = -0.0008059618890211334
R1 = 2.4857771759454126
RE = 21.54723134529601
IM2 = 31.799023222338658

_CACHED_NC = None
NWARM = 6  # PE/HAM warmup matmuls issued during the initial DMA wait
DR = mybir.MatmulPerfMode.DoubleRow


def _make_slots():
    """Chunk schedule: (tile 'T'|'U'|'V', w0, nw, combo, engine 'ACT'|'DVE').

    Three PSUM tiles rotate (T=3 banks, U=2, V=2, +1 bank for the output
    accumulator): every tile turnaround then has at least one full ACT
    chunk of cover, so the Sin->mains->Sin round-trip latency hides.  A
    combo-cycle is [T(3w), U(2w), V(2w)] = 7 windows; 4 window-groups x 8
    combos (cos/sin interleaved) cover w0-27, and the lone w28-30 block
    runs last as U(2w)/V(1w) chunks.  DVE (cos-poly) takes the T chunk of
    every cos combo cycle: 16 chunks, one per 2 cycles.
    """
    slots = []
    for p in range(3):
        # c4 first: the opening chunk is ACT (sin combo) so the first Sin
        # fires as soon as the first x chunks land
        for c in [4, 0, 5, 1, 6, 2, 7, 3]:
            w0 = 7 * p
            slots.append(("T", w0, 3, c, "DVE" if c < 4 else "ACT"))
            slots.append(("U", w0 + 3, 2, c, "ACT"))
            slots.append(("V", w0 + 5, 2, c, "ACT"))
    # last window-group also absorbs the lone w28-30 block as a second
    # 3-window T chunk per cycle ([T, U, Tb, V]: every T turnaround still
    # has an intervening ACT chunk).  Its last DVE chunk (c2) sits 4
    # cycles before the end so the poly tail and projections drain early.
    for c in [0, 4, 1, 5, 2, 6, 3, 7]:
        dve = c < 3
        slots.append(("T", 21, 3, c, "DVE" if dve else "ACT"))
        slots.append(("U", 24, 2, c, "ACT"))
        slots.append(("T", 28, 3, c, "ACT"))
        slots.append(("V", 26, 2, c, "ACT"))
    assert sum(nw for t, w0, nw, c, e in slots) == NWIN * NCOMBO
    return slots


class _Bacc(bacc.Bacc):
    """Bacc with a curated activation-table list: Exp/Ln resolve to the shared
    natural_log_exp_and_others set (one tail table load instead of two)."""

    def insert_act_table_loads(self):
        import bass_rust as _br
        from concourse.hw_specs import get_activation_tables

        has_activation = any(
            isinstance(i, mybir.InstActivation)
            for b in self.main_func.blocks
            for i in b.instructions
        )
        if not has_activation:
            return
        act = mybir.ActivationFunctionType
        tables = list(get_activation_tables(self.m.arch).items())
        names = [n for n, _ in tables]
        if "natural_log_exp_and_others" in names:
            keep = names.index("natural_log_exp_and_others")
            tables = [
                (
                    n,
                    fns
                    if i == keep
                    else {f for f in fns if f not in (act.Exp, act.Ln)},
                )
                for i, (n, fns) in enumerate(tables)
            ]
        _br.insert_act_table_loads(self, tables)


def _build_nc():
    nc = _Bacc()
    act = mybir.ActivationFunctionType
    alu = mybir.AluOpType

    x = nc.dram_tensor("x", [SEQ, B_LOCAL], FP8, kind="ExternalInput")  # xT
    wt = nc.dram_tensor("wt", [P, NCOMBO, 2, P], FP8, kind="ExternalInput")
    bias = nc.dram_tensor("bias", [P, NCOMBO], F32, kind="ExternalInput")
    biasd = nc.dram_tensor("biasd", [1, 4, P], BF16, kind="ExternalInput")
    wot = nc.dram_tensor("wot", [P, NCOMBO, OUT_DIM], BF16, kind="ExternalInput")
    bot = nc.dram_tensor("bot", [P, OUT_DIM], F32, kind="ExternalInput")
    y = nc.dram_tensor("y", [B_LOCAL, OUT_DIM], F32, kind="ExternalOutput")

    slots = _make_slots()

    with tile.TileContext(nc) as tc:
        with (
            tc.tile_pool(name="consts", bufs=1) as consts,
            tc.tile_pool(name="xt", bufs=1) as xtp,
            tc.tile_pool(name="ftT", bufs=8) as ftTp,
            tc.tile_pool(name="ftUV", bufs=8) as ftUVp,
            tc.tile_pool(name="dvv", bufs=2) as dvv,
            tc.tile_pool(name="dvu", bufs=2) as dvu,
            tc.tile_pool(name="dvf1", bufs=2) as dvf1,
            tc.tile_pool(name="dvt", bufs=2) as dvt,
            tc.tile_pool(name="dvt2", bufs=2) as dvt2,
            tc.tile_pool(name="dvf2", bufs=2) as dvf2,
            tc.tile_pool(name="tail", bufs=2) as tailp,
        ):
            # ---- warmup: pull the Sin table load to t~0 on ACT ----
            warm = consts.tile([P, 1], F32)
            nc.vector.memset(warm, 0.0)
            warm2 = consts.tile([P, 1], F32)
            nc.scalar.activation(warm2, warm, act.Sin, scale=1.0)
            # PE warmup operand
            wrm = consts.tile([P, B_LOCAL], BF16)
            nc.vector.memset(wrm, 0.0)
            # ones row for the K=1 bias matmuls
            ones = consts.tile([1, B_LOCAL], BF16)
            nc.vector.memset(ones, 1.0)

            # ---- constants + x.  Two DMA issue queues in parallel: the SP
            # HWDGE queue (650ns/issue) carries weights + late x groups; the
            # idle Pool SWDGE queue carries the first x groups + small
            # consts, so the first T chunk's data lands ~2.5us in. ----
            wt_sb = consts.tile([P, NCOMBO, 2, P], FP8)
            bias_sb = consts.tile([P, NCOMBO], F32)
            biasd_sb = consts.tile([1, 4, P], BF16)
            wot_sb = consts.tile([P, NCOMBO, OUT_DIM], BF16)
            bot_sb = consts.tile([P, OUT_DIM], F32)
            xt = xtp.tile([P, NCHUNK, B_LOCAL], FP8)

            def xgrp(queue, k0, gsz):
                queue.dma_start(
                    xt[:, k0 : k0 + gsz, :],
                    x[k0 * P : (k0 + gsz) * P, :].rearrange(
                        "(k p) b -> p k b", p=P
                    ),
                )

            nc.sync.dma_start(wt_sb[:, 4:5], wt[:, 4:5])  # first chunk = c4
            xgrp(nc.gpsimd, 0, 4)
            nc.sync.dma_start(bias_sb, bias[:, :])
            xgrp(nc.gpsimd, 4, 4)
            nc.sync.dma_start(wt_sb[:, 0:4], wt[:, 0:4])
            nc.gpsimd.dma_start(biasd_sb, biasd[:, :, :])
            xgrp(nc.sync, 8, 8)
            nc.gpsimd.dma_start(wot_sb, wot[:, :, :])
            nc.sync.dma_start(wt_sb[:, 5:], wt[:, 5:])
            nc.gpsimd.dma_start(bot_sb, bot[:, :])
            xgrp(nc.sync, 16, 8)
            xgrp(nc.sync, 24, 8)

            with (
                tc.tile_pool(name="psT", bufs=1, space="PSUM") as psTp,
                tc.tile_pool(name="psU", bufs=1, space="PSUM") as psUp,
                tc.tile_pool(name="psV", bufs=1, space="PSUM") as psVp,
                tc.tile_pool(name="fft", bufs=1, space="PSUM") as fftp,
            ):
                pstiles = {
                    "T": psTp.tile([P, 3, B_LOCAL], F32, tag="T", name="psT"),
                    "U": psUp.tile([P, 2, B_LOCAL], F32, tag="U", name="psU"),
                    "V": psVp.tile([P, 2, B_LOCAL], F32, tag="V", name="psV"),
                }
                fftb = fftp.tile([P, 512], F32, tag="fft")
                # zero the projection accumulator region (projections use
                # start=False so sub-bank groups never re-mark the bank's
                # pending-zero region)
                nc.vector.memset(fftb[:, : NBT * OUT_DIM], 0.0)

                if NWARM:
                    for _ in range(NWARM):
                        nc.tensor.matmul(
                            pstiles["T"][0:1, 0, :],
                            lhsT=wrm[:, 0:1],
                            rhs=wrm,
                            start=True,
                            stop=True,
                        )

                def emit_mains(slot):
                    tname, w0, nw, c, eng = slot
                    ps = pstiles[tname]
                    for wi in range(nw):
                        w = w0 + wi
                        if eng == "DVE":
                            # pre-add bias via K=1 bf16 matmul (b_cos row)
                            nc.tensor.matmul(
                                ps[:, wi, :],
                                lhsT=biasd_sb[0:1, c, :],
                                rhs=ones[0:1, :],
                                start=True,
                                stop=False,
                                skip_group_check=True,
                            )
                        nc.tensor.matmul(
                            ps[:, wi, :],
                            lhsT=wt_sb[:, c, :, :],
                            rhs=xt[:, w : w + 2, :],
                            start=(eng != "DVE"),
                            stop=True,
                            perf_mode=DR,
                            skip_group_check=True,
                        )
                    return ps

                def emit_consumer(item):
                    """ACT: full Sin.  DVE: only the PSUM-freeing TensorCopy —
                    the poly tail is emitted TAIL_DELAY slots later so that
                    ACT/PE sem waits (which quantize to the DVE instruction
                    count at emission time) resolve right after the copy."""
                    tname, w0, nw, c, eng = item["slot"]
                    ps = item["ps"]
                    if eng == "ACT":
                        pool = ftTp if tname == "T" else ftUVp
                        ft = pool.tile(
                            [P, 3 if tname == "T" else 2, B_LOCAL],
                            BF16,
                            tag="ft",
                        )
                        nc.scalar.activation(
                            ft[:, :nw, :],
                            ps[:, :nw, :],
                            act.Sin,
                            bias=bias_sb[:, c : c + 1],
                            scale=1.0,
                        )
                        item["ft"] = ft
                    else:
                        v = dvv.tile([P, 3, B_LOCAL], BF16, tag="v")
                        nc.vector.tensor_copy(v[:, :nw, :], ps[:, :nw, :])
                        item["v"] = v

                def emit_dve_tail(item):
                    # cos(v) = CC*(u-R1)*((u-RE)^2+IM2), u = v*v (SBUF bf16)
                    tname, w0, nw, c, eng = item["slot"]
                    vv = item["v"][:, :nw, :]
                    u = dvu.tile([P, 3, B_LOCAL], BF16, tag="u")
                    nc.vector.tensor_tensor(u[:, :nw, :], vv, vv, alu.mult)
                    uv = u[:, :nw, :]
                    f1 = dvf1.tile([P, 3, B_LOCAL], BF16, tag="f1")
                    nc.vector.tensor_scalar(
                        f1[:, :nw, :], uv, CC, -CC * R1, alu.mult, alu.add
                    )
                    t = dvt.tile([P, 3, B_LOCAL], BF16, tag="t")
                    nc.vector.tensor_scalar_sub(t[:, :nw, :], uv, RE)
                    t2 = dvt2.tile([P, 3, B_LOCAL], BF16, tag="t2")
                    nc.vector.tensor_tensor(
                        t2[:, :nw, :], t[:, :nw, :], t[:, :nw, :], alu.mult
                    )
                    f2 = dvf2.tile([P, 3, B_LOCAL], BF16, tag="f2")
                    nc.vector.tensor_scalar_add(f2[:, :nw, :], t2[:, :nw, :], IM2)
                    ft = ftTp.tile([P, 3, B_LOCAL], BF16, tag="ft")
                    nc.vector.tensor_tensor(
                        ft[:, :nw, :], f1[:, :nw, :], f2[:, :nw, :], alu.mult
                    )
                    item["ft"] = ft

                def emit_proj(item, last):
                    tname, w0, nw, c, eng = item["slot"]
                    ft = item["ft"]
                    for wi in range(nw):
                        for bt in range(NBT):
                            nc.tensor.matmul(
                                fftb[:, bt * OUT_DIM : (bt + 1) * OUT_DIM],
                                lhsT=ft[:, wi, bt * P : (bt + 1) * P],
                                rhs=wot_sb[:, c, :],
                                start=False,
                                stop=(last and wi == nw - 1),
                                skip_group_check=True,
                            )

                TAIL_DELAY = 3
                PROJ_DELAY = 6
                items = []
                for s, slot in enumerate(slots):
                    item = {"slot": slot, "ft": None, "v": None}
                    item["ps"] = emit_mains(slot)
                    emit_consumer(item)
                    items.append(item)
                    td = s - TAIL_DELAY
                    if td >= 0 and items[td]["slot"][4] == "DVE":
                        emit_dve_tail(items[td])
                    pd = s - PROJ_DELAY
                    if pd >= 0:
                        emit_proj(items[pd], last=False)
                for td in range(len(items) - TAIL_DELAY, len(items)):
                    if items[td]["slot"][4] == "DVE":
                        emit_dve_tail(items[td])
                for pd in range(len(items) - PROJ_DELAY, len(items)):
                    emit_proj(items[pd], last=(pd == len(items) - 1))

                # ---- tail: z = pf + b_out, batched log_softmax ----
                z_all = tailp.tile([P, NBT, OUT_DIM], F32, tag="z")
                for bt in range(NBT):
                    nc.vector.tensor_add(
                        z_all[:, bt, :],
                        fftb[:, bt * OUT_DIM : (bt + 1) * OUT_DIM],
                        bot_sb,
                    )
            e = tailp.tile([P, NBT, OUT_DIM], F32, tag="e")
            nc.scalar.activation(e, z_all, act.Exp)
            ssum = tailp.tile([P, NBT], F32, tag="ss")
            nc.vector.reduce_sum(ssum, e, axis=mybir.AxisListType.X)
            ls = tailp.tile([P, NBT], F32, tag="ls")
            nc.scalar.activation(ls, ssum, act.Ln)
            o = tailp.tile([P, NBT, OUT_DIM], F32, tag="o")
            nc.vector.tensor_tensor(
                o,
                z_all,
                ls[:, :, None].to_broadcast([P, NBT, OUT_DIM]),
                mybir.AluOpType.subtract,
            )
            nc.sync.dma_start(y.rearrange("(bt p) o -> p bt o", p=P), o)

    if not nc.is_finalized():
        nc.finalize()
    return nc


def _get_nc():
    global _CACHED_NC
    if _CACHED_NC is None:
        _CACHED_NC = _build_nc()
    return _CACHED_NC


def _make_in_maps(x, w_cos, b_cos, w_sin, b_sin, w_out, b_out):
    bf = ml_dtypes.bfloat16
    f8 = ml_dtypes.float8_e4m3
    x = np.asarray(x)
    w_cos, w_sin = np.asarray(w_cos), np.asarray(w_sin)
    b_cos, b_sin = np.asarray(b_cos), np.asarray(b_sin)
    w_out, b_out = np.asarray(w_out), np.asarray(b_out)
    # weights: [p, combo, ktile, m] fp8 (wt[p,c,j,m] = W[c*128+m, j*128+p])
    wt = np.concatenate([w_cos.T, w_sin.T], axis=1).reshape(2, P, NCOMBO, P)
    wt = np.ascontiguousarray(wt.transpose(1, 2, 0, 3)).astype(f8)
    # ACT bias: per-combo per-partition; fold pi/2 into cos (cos x = sin(x+pi/2))
    bias = np.empty((P, NCOMBO), np.float32)
    for mt in range(4):
        bias[:, mt] = b_cos[mt * P : (mt + 1) * P] + np.float32(np.pi / 2)
        bias[:, 4 + mt] = b_sin[mt * P : (mt + 1) * P]
    # DVE bias rows (plain b_cos, added in PSUM by K=1 matmul)
    biasd = b_cos.reshape(1, 4, P).astype(bf)
    # w_out.T with 1/31 mean folded in, chunked to [p, combo, o]
    wot = (w_out.T.astype(np.float64) / NWIN).astype(np.float32)
    wot = wot.reshape(NCOMBO, P, OUT_DIM).transpose(1, 0, 2).astype(bf)
    bot = np.broadcast_to(b_out.astype(np.float32), (P, OUT_DIM)).copy()

    in_maps = []
    for c in range(N_CORES):
        xs = x[c * B_LOCAL : (c + 1) * B_LOCAL, :]
        xt = np.ascontiguousarray(xs.T).astype(f8)  # [4096, 512]
        in_maps.append(
            {"x": xt, "wt": wt, "bias": bias, "biasd": biasd,
             "wot": wot, "bot": bot}
        )
    return in_maps


def run(inputs, trace=False, trace_cores=None):
    """Run the kernel; returns (y_full [4096,4] f32, BassKernelResults).

    Retries on transient device errors (the terminal occasionally reports
    NRT_EXEC_UNIT_UNRECOVERABLE after a prior crashed session and recovers
    on the next attempt)."""
    import time

    nc = _get_nc()
    in_maps = _make_in_maps(**inputs)
    last_err = None
    for attempt in range(3):
        try:
            res = run_bass_kernel_spmd(
                nc,
                in_maps,
                core_ids=list(range(N_CORES)),
                trace=trace,
                trace_cores=trace_cores,
            )
            y = np.concatenate([r["y"] for r in res.results], axis=0)
            return y, res
        except Exception as e:  # transient device wedge -> retry
            last_err = e
            if "UNRECOVERABLE" not in str(e) and "UNAVAILABLE" not in str(e):
                raise
            time.sleep(2.0)
    raise last_err


def kernel(**inputs):
    y, _ = run(inputs, trace=False)
    return y

